# revision 19
# baseline (speedup 1.0000x reference)
"""Multi-head self-attention (B=2, T=2048, C=1024, H=16, RoPE, causal) on 8 trn2 cores.

Sharding: data-parallel over batch (2) x tensor-parallel over head groups (4).
Core c handles batch c//4, heads (c%4)*4 .. +3.  Each core computes its
4 heads' attention output and a partial out-projection (contraction over its
256 head-dims); the host sums the 4 partials per batch.

Design (v4, fused software-pipelined schedule):
  - Stage p emits projection work for quarter p interleaved at matmul-chain
    granularity with attention for query-quarter p-1 and the out-projection
    for quarter p-2, so softmax exp (Activation engine) and PSUM evictions
    (DVE) overlap projection matmuls (PE) instead of serializing.
  - x and w_qkv are shipped bf16 (host-converted); x^T is built with bf16 PE
    transposes (1 cyc/row).  q^T/k^T/P/v are bf16 (rel err ~3.5e-3 total).
  - RoPE runs lane-aligned on [ev_all; od_all] chain pairs (the BIR verifier
    requires SBUF operands of elementwise ops to share a start partition);
    the [ev;od] staging is then permuted into head-pair-contiguous q^T/k^T
    by PE permutation-matrix matmuls (pm input) - no SBUF->SBUF merge DMAs.
  - Scores are computed transposed (S^T[k,q]) so softmax denominators come
    free from a ones-column in the AV stationary; exp is biased by -2 (exact
    cancellation in normalization) to keep headroom.
  - Engine assignment: Act = exp + v evictions; DVE = all other PSUM reads
    (GPSIMD cannot access PSUM); Pool = SBUF-only tri-masks; SP = all DMAs.
  - PSUM: one rotating 2-bank era-1 tag + 4-bank scores + 2-bank attention
    accumulators = exactly 8 banks.
  - x tiles for quarter p+1 prefetch mid-stage-p; era-1 chunks weave with
    att blocks proportionally (in-order engines follow emission order).
"""
import sys
import math

sys.path.insert(0, "/opt/trn_rl_repo")

import numpy as np

B, T, C, H, D = 2, 2048, 1024, 16, 64
HG = H // 4            # 4 heads per core
NCORES = 8
NKC = C // 128         # 8 contraction chunks
NQTR = T // 512        # 4 t-quarters
NKT = T // 128         # 16 k-tiles
ROPE_BASE = 10000.0

_BUILT = None


# ---------------------------------------------------------------------------
# Toolchain workaround: this walrus build accepts at most ONE semaphore wait
# per instruction.  Tile's exit drain carries one wait per outstanding proc,
# and stage-1B can attach several waits to compute/DMA instructions.  We
# (a) replace the exit drain with a chain of single-wait drains, and
# (b) post-process the module, hoisting extra waits onto same-engine nops.
# ---------------------------------------------------------------------------

def _apply_tile_patch():
    import bass_rust
    import concourse.tile as tile
    from concourse.vector_clock import ScopedClock

    def _patched_drain_and_barrier(self, tick_clock, wait_clock):
        nc = self.nc
        probe = nc.sync.drain()
        wait_clock.add_sem_waits(probe.ins, ScopedClock({None: tick_clock.global_clock}))
        si = probe.ins.sync_info
        waits = list(si.on_wait) if si is not None else []
        probe.ins.sync_info = None
        name2sem = {s.name: s for s in wait_clock.sems.allocated().values()}
        for w in waits:
            d = nc.sync.drain()
            bass_rust.wait_op(d.ins, name2sem[w.ant_name], w.wait_value, "sem-ge", False)
        nc.all_engine_barrier()
        popped = nc._tile_sem_poison_stack.pop()
        assert popped is self._sem_poison
        nc.clear_and_free_semaphores(list(self.sems.allocated().values()))
        nc.all_engine_barrier()

    tile.TileContext._drain_and_barrier = _patched_drain_and_barrier


def _split_multi_waits(nc):
    import bass_rust
    import concourse.mybir as mybir

    ctr = 0
    for fn in nc.m.functions:
        for blk in fn.blocks:
            il = blk.instructions
            new = []
            changed = False
            for inst in il:
                si = inst.sync_info
                waits = list(si.on_wait) if si is not None else []
                if len(waits) > 1:
                    changed = True
                    for w in waits[:-1]:
                        nop = mybir.InstNoOp(name=f"I-waitsplit-{ctr}", ins=[], outs=[])
                        ctr += 1
                        nop.engine = inst.engine
                        nop.sync_info = bass_rust.SyncInfo(on_wait=[w], on_update=[])
                        new.append(nop)
                    inst.sync_info = bass_rust.SyncInfo(
                        on_wait=[waits[-1]], on_update=list(si.on_update)
                    )
                new.append(inst)
            if changed:
                blk.instructions = new


def _weave(primary, secondary):
    """Interleave two emitter lists proportionally; call every closure once.

    primary items are emitted at least as early as their proportional
    position; used to mix era-1 chunks (primary) into att blocks.
    """
    na, nb = len(primary), len(secondary)
    ia = ib = 0
    while ia < na or ib < nb:
        if ib >= nb or (ia < na and ia * max(nb, 1) <= ib * max(na, 1)):
            primary[ia]()
            ia += 1
        else:
            secondary[ib]()
            ib += 1


# ---------------------------------------------------------------------------
# Kernel builder (per-core program; identical on all 8 cores)
# ---------------------------------------------------------------------------

def build_nc(split_waits=True, loop_iters=None, phases=(1, 2)):
    _apply_tile_patch()
    import concourse.bass as bass
    import concourse.tile as tile
    import concourse.mybir as mybir
    from concourse.masks import make_identity
    from contextlib import nullcontext

    dt = mybir.dt
    f32, f32r = dt.float32, dt.float32r
    f8 = dt.float8e4
    bf16 = dt.bfloat16
    DR = mybir.MatmulPerfMode.DoubleRow
    Exp = mybir.ActivationFunctionType.Exp
    MUL, SUB, ADD = (mybir.AluOpType.mult, mybir.AluOpType.subtract,
                     mybir.AluOpType.add)

    nc = bass.Bass()
    x_d = nc.dram_tensor("x", [T, C], bf16, kind="ExternalInput")
    w_d = nc.dram_tensor("w", [C, 768], bf16, kind="ExternalInput")
    wo_d = nc.dram_tensor("wo", [256, C], f32, kind="ExternalInput")
    cs_d = nc.dram_tensor("cs", [128, T], f32, kind="ExternalInput")
    sn_d = nc.dram_tensor("sn", [128, T], f32, kind="ExternalInput")
    tri_d = nc.dram_tensor("tri", [128, 128], bf16, kind="ExternalInput")
    pm_d = nc.dram_tensor("pm", [128, 4, 128], bf16, kind="ExternalInput")
    y_d = nc.dram_tensor("y", [T, C], f32, kind="ExternalOutput")

    with tile.TileContext(nc) as tc:
      loop_cm = (tc.For_i(0, loop_iters, 1,
                          hint_engines=(mybir.EngineType.PE, mybir.EngineType.Activation,
                                        mybir.EngineType.DVE, mybir.EngineType.SP,
                                        mybir.EngineType.Pool))
                 if loop_iters else nullcontext())
      with loop_cm:
        with (
            tc.tile_pool(name="persist", bufs=1) as persist,
            tc.tile_pool(name="qkT", bufs=1) as qkT_pool,
            tc.tile_pool(name="asb", bufs=1) as asb_pool,
            tc.tile_pool(name="w", bufs=1) as w_pool,
            tc.tile_pool(name="xload", bufs=8) as x_pool,
            tc.tile_pool(name="xT", bufs=2) as xT_pool,
            tc.tile_pool(name="rope", bufs=2) as rope_pool,
            tc.tile_pool(name="pt", bufs=4) as pt_pool,
            tc.tile_pool(name="nrm", bufs=2) as nrm_pool,
            tc.tile_pool(name="yout", bufs=3) as y_pool,
            tc.tile_pool(name="ps_e1", bufs=2, space="PSUM") as ps_e1_pool,
            tc.tile_pool(name="ps_s", bufs=2, space="PSUM") as ps_s_pool,
            tc.tile_pool(name="ps_o", bufs=1, space="PSUM") as ps_o_pool,
        ):
            # per-head-contiguous rotated q^T/k^T: tile [128, T] = 2 heads.
            # bf16: halves SBUF + enables DVE 2x mode for the RoPE writes.
            qT = [qkT_pool.tile([128, T], bf16, tag=f"qT{i}", name=f"qT{i}") for i in range(2)]
            kT = [qkT_pool.tile([128, T], bf16, tag=f"kT{i}", name=f"kT{i}") for i in range(2)]
            # v in (t, d) layout + ones column per head slot
            v_sb = persist.tile([128, NKT, 4 * 65], bf16, tag="v")
            wo_sb = persist.tile([128, 2, C], f32r, tag="wo")
            tri_sb = persist.tile([128, 128], bf16, tag="tri")
            cs_sb = persist.tile([128, T], f32, tag="cs")
            sn_sb = persist.tile([128, T], f32, tag="sn")
            a_sb = [asb_pool.tile([128, T], f32r, tag=f"a{i}", name=f"a{i}") for i in range(2)]
            w_sb = w_pool.tile([128, NKC, 768], bf16, tag="w")
            pm_sb = w_pool.tile([128, 4, 128], bf16, tag="pm")
            ident = w_pool.tile([128, 128], bf16, tag="ident")
            make_identity(nc, ident[:])
            identr = ident[:]
            ebias_t = w_pool.tile([128, 1], f32, tag="ebias")
            nc.gpsimd.memset(ebias_t[:], -2.0)
            ebias = ebias_t[:]

            v4 = v_sb[:].rearrange("p kt (h c) -> p kt h c", h=4)
            # ones columns of v (col 64 of each 68-wide head slot)
            nc.gpsimd.memset(v4[:, :, :, 64:65], 1.0)

            # ---------------- era-1 emitters for quarter p ----------------
            # w column layout (host-permuted): [QE | QO | KE | KO | V] where
            # QE = evens of all 4 heads (4x32), QO = odds, etc.  RoPE math is
            # lane-aligned on the ev/od chain pair; the PE then permutes the
            # [ev_all; od_all] staging into head-contiguous q^T/k^T via
            # permutation-matrix matmuls (perm tables shipped in pm).
            xstore = {}

            def _load_x_tiles(p):
                xts = []
                for tl in range(4):
                    xt = x_pool.tile([128, C], bf16, tag="x", name=f"x{p}{tl}")
                    t0 = p * 512 + tl * 128
                    for h in range(2):
                        nc.sync.dma_start(
                            xt[:, h * 512:(h + 1) * 512],
                            x_d[t0:t0 + 128, h * 512:(h + 1) * 512])
                    xts.append(xt)
                xstore[p] = xts

            def era1_chunks(p):
                state = {}

                def load_x(p=p):
                    if p == 0:
                        _load_x_tiles(0)
                        for kc in range(NKC):
                            nc.sync.dma_start(w_sb[:, kc, :],
                                              w_d[kc * 128:(kc + 1) * 128, :])
                        nc.sync.dma_start(cs_sb[:], cs_d[:])
                        nc.sync.dma_start(sn_sb[:], sn_d[:])
                        nc.sync.dma_start(pm_sb[:], pm_d[:])
                    elif p == 1:
                        nc.sync.dma_start(
                            wo_sb[:],
                            wo_d[:].rearrange("(kc p) c -> p kc c", p=128).bitcast(f32r))
                        nc.sync.dma_start(tri_sb[:], tri_d[:])
                    state["xts"] = xstore[p]
                    state["xT"] = xT_pool.tile([128, NKC, 512], bf16, tag="xT",
                                               name=f"xT{p}")

                def prefetch_x(p=p):
                    if p + 1 < NQTR:
                        _load_x_tiles(p + 1)

                def transpose_tl(tl, p=p):
                    def emit():
                        xt = state["xts"][tl]
                        xT_q = state["xT"]
                        for kc4 in range(2):
                            ptr = ps_e1_pool.tile([128, 4, 128], bf16, tag="e1",
                                                  name=f"tr{p}{tl}{kc4}")
                            for j in range(4):
                                kc = 4 * kc4 + j
                                nc.tensor.transpose(
                                    ptr[:, j, :], xt[:, kc * 128:(kc + 1) * 128], identr)
                            nc.vector.tensor_copy(
                                xT_q[:, 4 * kc4:4 * kc4 + 4, tl * 128:(tl + 1) * 128],
                                ptr[:])
                    return emit

                def vproj_tl(tl, p=p):
                    def emit():
                        xT_q = state["xT"]
                        psv = ps_e1_pool.tile([128, 256], f32, tag="e1",
                                              name=f"v{p}{tl}")
                        for kc in range(NKC):
                            nc.tensor.matmul(psv[:], xT_q[:, kc, tl * 128:(tl + 1) * 128],
                                             w_sb[:, kc, 512:768],
                                             start=(kc == 0), stop=(kc == NKC - 1))
                        kt = p * 4 + tl
                        nc.scalar.copy(
                            v4[:, kt, :, 0:64],
                            psv[:].rearrange("p (h c) -> p h c", h=4))
                    return emit

                def qk_even(kind, p=p):
                    def emit():
                        xT_q = state["xT"]
                        ci = 0 if kind == "q" else 2
                        ps = ps_e1_pool.tile([128, 512], f32, tag="e1",
                                             name=f"qe{p}{kind}")
                        for kc in range(NKC):
                            nc.tensor.matmul(ps[:], w_sb[:, kc, ci * 128:(ci + 1) * 128],
                                             xT_q[:, kc, :],
                                             start=(kc == 0), stop=(kc == NKC - 1))
                        cs_c = cs_sb[:, p * 512:(p + 1) * 512]
                        sn_c = sn_sb[:, p * 512:(p + 1) * 512]
                        t1 = rope_pool.tile([128, 512], bf16, tag="t1", name=f"t1_{p}{kind}")
                        t1b = rope_pool.tile([128, 512], bf16, tag="t1b", name=f"t1b_{p}{kind}")
                        nc.vector.tensor_tensor(t1[:], ps[:], cs_c, MUL)
                        nc.vector.tensor_tensor(t1b[:], ps[:], sn_c, MUL)
                        state["t1"], state["t1b"] = t1, t1b
                    return emit

                def qk_odd(kind, p=p):
                    def emit():
                        xT_q = state["xT"]
                        ci = 1 if kind == "q" else 3
                        ps = ps_e1_pool.tile([128, 512], f32, tag="e1",
                                             name=f"qo{p}{kind}")
                        for kc in range(NKC):
                            nc.tensor.matmul(ps[:], w_sb[:, kc, ci * 128:(ci + 1) * 128],
                                             xT_q[:, kc, :],
                                             start=(kc == 0), stop=(kc == NKC - 1))
                        cs_c = cs_sb[:, p * 512:(p + 1) * 512]
                        sn_c = sn_sb[:, p * 512:(p + 1) * 512]
                        t2 = rope_pool.tile([128, 512], bf16, tag="t2", name=f"t2_{p}{kind}")
                        t2b = rope_pool.tile([128, 512], bf16, tag="t2b", name=f"t2b_{p}{kind}")
                        nc.vector.tensor_tensor(t2[:], ps[:], sn_c, MUL)
                        nc.vector.tensor_tensor(t2b[:], ps[:], cs_c, MUL)
                        ev = rope_pool.tile([128, 512], bf16, tag="ev", name=f"ev_{p}{kind}")
                        od = rope_pool.tile([128, 512], bf16, tag="od", name=f"od_{p}{kind}")
                        nc.vector.tensor_tensor(ev[:], state["t1"][:], t2[:], SUB)
                        nc.vector.tensor_tensor(od[:], state["t1b"][:], t2b[:], ADD)
                        state["ev"], state["od"] = ev, od
                    return emit

                def qk_merge(kind, p=p):
                    def emit():
                        ev, od = state["ev"], state["od"]
                        dst = qT if kind == "q" else kT
                        cl = slice(p * 512, (p + 1) * 512)
                        for hp in range(2):
                            psM = ps_e1_pool.tile([128, 512], f32, tag="e1",
                                                  name=f"mg{p}{kind}{hp}")
                            nc.tensor.matmul(psM[:], pm_sb[:, 2 * hp, :], ev[:],
                                             start=True, stop=False, skip_group_check=True)
                            nc.tensor.matmul(psM[:], pm_sb[:, 2 * hp + 1, :], od[:],
                                             start=False, stop=True, skip_group_check=True)
                            nc.vector.tensor_copy(dst[hp][:, cl], psM[:])
                    return emit

                chunks = [load_x]
                for tl in range(2):
                    chunks.append(transpose_tl(tl))
                chunks.append(transpose_tl(2))
                chunks.append(vproj_tl(0))
                chunks.append(transpose_tl(3))
                chunks.append(prefetch_x)
                chunks.append(vproj_tl(1))
                chunks.append(vproj_tl(2))
                chunks.append(vproj_tl(3))
                for kind in ("q", "k"):
                    chunks.append(qk_even(kind))
                    chunks.append(qk_odd(kind))
                    chunks.append(qk_merge(kind))
                return chunks

            # ---------------- attention emitters for query-quarter qc ----------------
            def att_blocks(qc):
                blocks = []
                nkt_q = (qc + 1) * 4
                ps_o_tiles = {}

                def sblock(hp, ki2, qc=qc):
                    def emit():
                        nkt_q_ = (qc + 1) * 4
                        ki0 = 2 * ki2
                        if ki2 == 0:
                            ps_o_tiles[hp] = [
                                ps_o_pool.tile([65, 512], f32, tag=f"o{hh}",
                                               name=f"pso{qc}{hp}{hh}")
                                for hh in range(2)]
                        ps_o = ps_o_tiles[hp]
                        ps_s2 = [ps_s_pool.tile([128, 1024], f32, tag="s",
                                                name=f"ps_s{qc}{hp}{ki2}{hh}")
                                 for hh in range(2)]
                        for half in range(2):
                            ki = ki0 + half
                            soff = max(0, ki * 128 - qc * 512) if ki // 4 == qc else 0
                            for hh in range(2):
                                r0 = hh * 64
                                nc.tensor.matmul(
                                    ps_s2[hh][:, half * 512 + soff:(half + 1) * 512],
                                    kT[hp][r0:r0 + 64, ki * 128:(ki + 1) * 128],
                                    qT[hp][r0:r0 + 64, qc * 512 + soff:(qc + 1) * 512],
                                    start=True, stop=True, skip_group_check=True)
                        diag_pair = (ki0 // 4 == qc) or ((ki0 + 1) // 4 == qc)
                        pts = []
                        for hh in range(2):
                            ps_s = ps_s2[hh]
                            ps_sv = ps_s[:].rearrange("p (two f) -> p two f", two=2)
                            pt = pt_pool.tile([128, 2, 512], bf16, tag="pt",
                                              name=f"pt{qc}{hp}{ki2}{hh}")
                            pts.append(pt)
                            if not diag_pair:
                                nc.scalar.activation(pt[:], ps_sv, Exp, scale=0.125, bias=ebias)
                            else:
                                for half in range(2):
                                    ki = ki0 + half
                                    off = ki * 128 - qc * 512
                                    if ki // 4 == qc:   # diagonal tile
                                        # cols < off are never read by the AV
                                        # matmul (it starts at soff), so only
                                        # exp the live region + tri-mask the
                                        # 128-wide diagonal band.
                                        nc.scalar.activation(
                                            pt[:, half, off:512],
                                            ps_sv[:, half, off:512], Exp, scale=0.125, bias=ebias)
                                        nc.gpsimd.tensor_tensor(
                                            pt[:, half, off:off + 128],
                                            pt[:, half, off:off + 128],
                                            tri_sb[:], MUL)
                                    else:
                                        nc.scalar.activation(
                                            pt[:, half, :],
                                            ps_sv[:, half, :], Exp, scale=0.125, bias=ebias)
                        for hh in range(2):
                            h = hp * 2 + hh
                            for half in range(2):
                                ki = ki0 + half
                                soff = (max(0, ki * 128 - qc * 512)
                                        if (ki // 4 == qc and ki != 0) else 0)
                                nc.tensor.matmul(
                                    ps_o[hh][:, soff:512],
                                    v4[:, ki, h, 0:65],
                                    pts[hh][:, half, soff:512],
                                    start=(ki == 0), stop=(ki == nkt_q_ - 1),
                                    skip_group_check=True)
                    return emit

                def norm(hp, qc=qc):
                    def emit():
                        for hh in range(2):
                            ps_o = ps_o_tiles[hp][hh]
                            rrow = nrm_pool.tile([1, 512], f32, tag="rrow",
                                                 name=f"rr{qc}{hp}{hh}")
                            nc.vector.reciprocal(rrow[:], ps_o[64:65, :])
                            bsum = nrm_pool.tile([64, 512], f32, tag="bsum",
                                                 name=f"bs{qc}{hp}{hh}")
                            nc.sync.dma_start(
                                bsum[:], rrow[0:1, None, :].to_broadcast([1, 64, 512]))
                            nc.vector.tensor_tensor(
                                a_sb[hp][hh * 64:(hh + 1) * 64, qc * 512:(qc + 1) * 512],
                                ps_o[0:64, :], bsum[:], MUL)
                    return emit

                def outproj(tl, qc=qc):
                    def emit():
                        ti = qc * 4 + tl
                        for ncol in range(2):
                            psy = ps_e1_pool.tile([128, 512], f32, tag="e1",
                                                  name=f"y{qc}{tl}{ncol}")
                            for kc2 in range(2):
                                nc.tensor.matmul(psy[:], a_sb[kc2][:, ti * 128:(ti + 1) * 128],
                                                 wo_sb[:, kc2, ncol * 512:(ncol + 1) * 512],
                                                 start=(kc2 == 0), stop=(kc2 == 1))
                            yt = y_pool.tile([128, 512], f32, tag="yt",
                                             name=f"yt{qc}{tl}{ncol}")
                            nc.vector.tensor_copy(yt[:], psy[:])
                            nc.sync.dma_start(
                                y_d[ti * 128:(ti + 1) * 128, ncol * 512:(ncol + 1) * 512],
                                yt[:])
                    return emit

                for hp in range(2):
                    for ki2 in range(nkt_q // 2):
                        blocks.append(sblock(hp, ki2))
                    blocks.append(norm(hp))
                outs = [outproj(tl) for tl in range(4)]
                return blocks, outs

            # ---------------- fused stage schedule ----------------
            # stage s runs: era1(s), attention(s-1), out-projection(s-2)
            pending_outs = []
            for stage in range(NQTR + 2):
                e1 = era1_chunks(stage) if (stage < NQTR and 1 in phases) else []
                if stage >= 1 and stage <= NQTR and 2 in phases:
                    a2, outs = att_blocks(stage - 1)
                else:
                    a2, outs = [], []
                # spread deferred out-projection blocks (pure PE filler)
                # evenly through the att blocks instead of bunching at front
                if pending_outs and a2:
                    merged = []
                    step = max(1, len(a2) // len(pending_outs))
                    oi = 0
                    for bi, b in enumerate(a2):
                        merged.append(b)
                        if bi % step == step - 1 and oi < len(pending_outs):
                            merged.append(pending_outs[oi])
                            oi += 1
                    merged.extend(pending_outs[oi:])
                    a2 = merged
                else:
                    a2 = pending_outs + a2
                pending_outs = outs
                if e1:
                    e1[0]()      # x DMAs (and stage-0 weight loads) first
                    e1 = e1[1:]
                _weave(e1, a2)

    if split_waits:
        _split_multi_waits(nc)
    return nc


# ---------------------------------------------------------------------------
# Host-side sharding / gather
# ---------------------------------------------------------------------------

def _rope_tables():
    inv_freq = (1.0 / (ROPE_BASE ** (np.arange(0, D, 2, dtype=np.float32) / D))).astype(np.float32)
    ang = (np.arange(T, dtype=np.float32)[:, None] * inv_freq[None, :]).astype(np.float32)  # (T, 32)
    cos, sin = np.cos(ang), np.sin(ang)
    idx = np.arange(128) % 32
    return np.ascontiguousarray(cos[:, idx].T), np.ascontiguousarray(sin[:, idx].T)  # (128, T)


def _perm_cols(g):
    """w_qkv column order for core group g: [QE|QO|KE|KO|V] (ev/od chains)."""
    cols = []
    for base, par in ((0, 0), (0, 1), (C, 0), (C, 1)):      # QE, QO, KE, KO
        for hl in range(4):
            hg = g * 4 + hl
            for i in range(32):
                cols.append(base + hg * 64 + 2 * i + par)
    for hl in range(4):
        hg = g * 4 + hl
        for d_ in range(64):
            cols.append(2 * C + hg * 64 + d_)
    return np.asarray(cols)


def _perm_tables():
    """pm[:, 2*hp+eo, :]: permutation stationaries mapping ev/od staging rows
    (4 heads x 32) onto head-pair tile rows [hA ev|hA od|hB ev|hB od]."""
    pm = np.zeros((128, 4, 128), np.float32)
    for hp in range(2):
        for j in range(128):
            blk, i = j // 32, j % 32
            h = 2 * hp + (1 if blk >= 2 else 0)
            src_row = h * 32 + i
            if blk % 2 == 0:     # ev rows of the pair tile
                pm[src_row, 2 * hp, j] = 1.0
            else:                # od rows
                pm[src_row, 2 * hp + 1, j] = 1.0
    return pm


def _perm_wo_rows(g):
    # O^T rows follow v's ORIGINAL dim order (RoPE only permutes q/k dims),
    # so the out-projection rows are just this head group's contiguous block.
    return np.arange(g * 256, (g + 1) * 256)


def make_in_maps(x, w_qkv, w_out):
    x = np.asarray(x, np.float32)
    w_qkv = np.asarray(w_qkv, np.float32)
    w_out = np.asarray(w_out, np.float32)
    import concourse.mybir as mybir
    cs, sn = _rope_tables()
    f8np = mybir.dt.np(mybir.dt.float8e4)
    bf16np = mybir.dt.np(mybir.dt.bfloat16)
    tri = np.tril(np.ones((128, 128), np.float32)).T.astype(bf16np)  # tri[k,q]=1 iff q>=k
    pm = _perm_tables().astype(bf16np)
    in_maps = []
    for c in range(NCORES):
        b, g = c // 4, c % 4
        in_maps.append({
            "x": np.ascontiguousarray(x[b]).astype(bf16np),
            "w": np.ascontiguousarray(w_qkv[:, _perm_cols(g)]).astype(bf16np),
            "wo": np.ascontiguousarray(w_out[_perm_wo_rows(g), :]),
            "cs": cs, "sn": sn, "tri": tri, "pm": pm,
        })
    return in_maps


def kernel(x, w_qkv, w_out):
    global _BUILT
    from concourse.bass_utils import run_bass_kernel_spmd

    if _BUILT is None:
        _BUILT = build_nc()
    in_maps = make_in_maps(x, w_qkv, w_out)
    res = run_bass_kernel_spmd(_BUILT, in_maps, core_ids=list(range(NCORES)))
    out = np.zeros((B, T, C), np.float32)
    for c in range(NCORES):
        out[c // 4] += res.results[c]["y"]
    return out


# revision 21
# speedup vs baseline: 1.0417x; 1.0417x over previous
"""Multi-head self-attention (B=2, T=2048, C=1024, H=16, RoPE, causal) on 8 trn2 cores.

Sharding: data-parallel over batch (2) x tensor-parallel over head groups (4).
Core c handles batch c//4, heads (c%4)*4 .. +3.  Each core computes its
4 heads' attention output and a partial out-projection (contraction over its
256 head-dims); the host sums the 4 partials per batch.

Design (v4, fused software-pipelined schedule):
  - Stage p emits projection work for quarter p interleaved at matmul-chain
    granularity with attention for query-quarter p-1 and the out-projection
    for quarter p-2, so softmax exp (Activation engine) and PSUM evictions
    (DVE) overlap projection matmuls (PE) instead of serializing.
  - x and w_qkv are shipped bf16 (host-converted); x^T is built with bf16 PE
    transposes (1 cyc/row).  q^T/k^T/P/v are bf16 (rel err ~3.5e-3 total).
  - RoPE runs lane-aligned on [ev_all; od_all] chain pairs (the BIR verifier
    requires SBUF operands of elementwise ops to share a start partition);
    the [ev;od] staging is then permuted into head-pair-contiguous q^T/k^T
    by PE permutation-matrix matmuls (pm input) - no SBUF->SBUF merge DMAs.
  - Scores are computed transposed (S^T[k,q]) so softmax denominators come
    free from a ones-column in the AV stationary; exp is biased by -2 (exact
    cancellation in normalization) to keep headroom.
  - Engine assignment: Act = exp + v evictions; DVE = all other PSUM reads
    (GPSIMD cannot access PSUM); Pool = SBUF-only tri-masks; SP = all DMAs.
  - PSUM: one rotating 2-bank era-1 tag + 4-bank scores + 2-bank attention
    accumulators = exactly 8 banks.
  - x tiles for quarter p+1 prefetch mid-stage-p; era-1 chunks weave with
    att blocks proportionally (in-order engines follow emission order).
"""
import sys
import math

sys.path.insert(0, "/opt/trn_rl_repo")

import numpy as np

B, T, C, H, D = 2, 2048, 1024, 16, 64
HG = H // 4            # 4 heads per core
NCORES = 8
NKC = C // 128         # 8 contraction chunks
NQTR = T // 512        # 4 t-quarters
NKT = T // 128         # 16 k-tiles
ROPE_BASE = 10000.0

_BUILT = None


# ---------------------------------------------------------------------------
# Toolchain workaround: this walrus build accepts at most ONE semaphore wait
# per instruction.  Tile's exit drain carries one wait per outstanding proc,
# and stage-1B can attach several waits to compute/DMA instructions.  We
# (a) replace the exit drain with a chain of single-wait drains, and
# (b) post-process the module, hoisting extra waits onto same-engine nops.
# ---------------------------------------------------------------------------

def _apply_tile_patch():
    import bass_rust
    import concourse.tile as tile
    from concourse.vector_clock import ScopedClock

    def _patched_drain_and_barrier(self, tick_clock, wait_clock):
        nc = self.nc
        probe = nc.sync.drain()
        wait_clock.add_sem_waits(probe.ins, ScopedClock({None: tick_clock.global_clock}))
        si = probe.ins.sync_info
        waits = list(si.on_wait) if si is not None else []
        probe.ins.sync_info = None
        name2sem = {s.name: s for s in wait_clock.sems.allocated().values()}
        for w in waits:
            d = nc.sync.drain()
            bass_rust.wait_op(d.ins, name2sem[w.ant_name], w.wait_value, "sem-ge", False)
        nc.all_engine_barrier()
        popped = nc._tile_sem_poison_stack.pop()
        assert popped is self._sem_poison
        nc.clear_and_free_semaphores(list(self.sems.allocated().values()))
        nc.all_engine_barrier()

    tile.TileContext._drain_and_barrier = _patched_drain_and_barrier


def _split_multi_waits(nc):
    import bass_rust
    import concourse.mybir as mybir

    ctr = 0
    for fn in nc.m.functions:
        for blk in fn.blocks:
            il = blk.instructions
            new = []
            changed = False
            for inst in il:
                si = inst.sync_info
                waits = list(si.on_wait) if si is not None else []
                if len(waits) > 1:
                    changed = True
                    for w in waits[:-1]:
                        nop = mybir.InstNoOp(name=f"I-waitsplit-{ctr}", ins=[], outs=[])
                        ctr += 1
                        nop.engine = inst.engine
                        nop.sync_info = bass_rust.SyncInfo(on_wait=[w], on_update=[])
                        new.append(nop)
                    inst.sync_info = bass_rust.SyncInfo(
                        on_wait=[waits[-1]], on_update=list(si.on_update)
                    )
                new.append(inst)
            if changed:
                blk.instructions = new


def _weave(primary, secondary):
    """Interleave two emitter lists proportionally; call every closure once.

    primary items are emitted at least as early as their proportional
    position; used to mix era-1 chunks (primary) into att blocks.
    """
    na, nb = len(primary), len(secondary)
    ia = ib = 0
    while ia < na or ib < nb:
        if ib >= nb or (ia < na and ia * max(nb, 1) <= ib * max(na, 1)):
            primary[ia]()
            ia += 1
        else:
            secondary[ib]()
            ib += 1


# ---------------------------------------------------------------------------
# Kernel builder (per-core program; identical on all 8 cores)
# ---------------------------------------------------------------------------

def build_nc(split_waits=True, loop_iters=None, phases=(1, 2)):
    _apply_tile_patch()
    import concourse.bass as bass
    import concourse.tile as tile
    import concourse.mybir as mybir
    from concourse.masks import make_identity
    from contextlib import nullcontext

    dt = mybir.dt
    f32, f32r = dt.float32, dt.float32r
    f8 = dt.float8e4
    bf16 = dt.bfloat16
    DR = mybir.MatmulPerfMode.DoubleRow
    Exp = mybir.ActivationFunctionType.Exp
    MUL, SUB, ADD = (mybir.AluOpType.mult, mybir.AluOpType.subtract,
                     mybir.AluOpType.add)

    nc = bass.Bass()
    x_d = nc.dram_tensor("x", [T, C], bf16, kind="ExternalInput")
    w_d = nc.dram_tensor("w", [C, 768], bf16, kind="ExternalInput")
    wo_d = nc.dram_tensor("wo", [256, C], f32, kind="ExternalInput")
    cs_d = nc.dram_tensor("cs", [128, T], f32, kind="ExternalInput")
    sn_d = nc.dram_tensor("sn", [128, T], f32, kind="ExternalInput")
    tri_d = nc.dram_tensor("tri", [128, 128], bf16, kind="ExternalInput")
    pm_d = nc.dram_tensor("pm", [128, 4, 128], bf16, kind="ExternalInput")
    y_d = nc.dram_tensor("y", [T, C], f32, kind="ExternalOutput")

    with tile.TileContext(nc) as tc:
      loop_cm = (tc.For_i(0, loop_iters, 1,
                          hint_engines=(mybir.EngineType.PE, mybir.EngineType.Activation,
                                        mybir.EngineType.DVE, mybir.EngineType.SP,
                                        mybir.EngineType.Pool))
                 if loop_iters else nullcontext())
      with loop_cm:
        with (
            tc.tile_pool(name="persist", bufs=1) as persist,
            tc.tile_pool(name="qkT", bufs=1) as qkT_pool,
            tc.tile_pool(name="asb", bufs=1) as asb_pool,
            tc.tile_pool(name="w", bufs=1) as w_pool,
            tc.tile_pool(name="xload", bufs=8) as x_pool,
            tc.tile_pool(name="xT", bufs=3) as xT_pool,
            tc.tile_pool(name="rope", bufs=3) as rope_pool,
            tc.tile_pool(name="pt", bufs=6) as pt_pool,
            tc.tile_pool(name="nrm", bufs=3) as nrm_pool,
            tc.tile_pool(name="yout", bufs=4) as y_pool,
            tc.tile_pool(name="ps_e1", bufs=2, space="PSUM") as ps_e1_pool,
            tc.tile_pool(name="ps_s", bufs=2, space="PSUM") as ps_s_pool,
            tc.tile_pool(name="ps_o", bufs=1, space="PSUM") as ps_o_pool,
        ):
            # per-head-contiguous rotated q^T/k^T: tile [128, T] = 2 heads.
            # bf16: halves SBUF + enables DVE 2x mode for the RoPE writes.
            qT = [qkT_pool.tile([128, T], bf16, tag=f"qT{i}", name=f"qT{i}") for i in range(2)]
            kT = [qkT_pool.tile([128, T], bf16, tag=f"kT{i}", name=f"kT{i}") for i in range(2)]
            # v in (t, d) layout + ones column per head slot
            v_sb = persist.tile([128, NKT, 4 * 65], bf16, tag="v")
            wo_sb = persist.tile([128, 2, C], f32r, tag="wo")
            tri_sb = persist.tile([128, 128], bf16, tag="tri")
            cs_sb = persist.tile([128, T], f32, tag="cs")
            sn_sb = persist.tile([128, T], f32, tag="sn")
            a_sb = [asb_pool.tile([128, T], f32r, tag=f"a{i}", name=f"a{i}") for i in range(2)]
            w_sb = w_pool.tile([128, NKC, 768], bf16, tag="w")
            pm_sb = w_pool.tile([128, 4, 128], bf16, tag="pm")
            ident = w_pool.tile([128, 128], bf16, tag="ident")
            make_identity(nc, ident[:])
            identr = ident[:]
            ebias_t = w_pool.tile([128, 1], f32, tag="ebias")
            nc.gpsimd.memset(ebias_t[:], -2.0)
            ebias = ebias_t[:]

            v4 = v_sb[:].rearrange("p kt (h c) -> p kt h c", h=4)
            # ones columns of v (col 64 of each 68-wide head slot)
            nc.gpsimd.memset(v4[:, :, :, 64:65], 1.0)

            # ---------------- era-1 emitters for quarter p ----------------
            # w column layout (host-permuted): [QE | QO | KE | KO | V] where
            # QE = evens of all 4 heads (4x32), QO = odds, etc.  RoPE math is
            # lane-aligned on the ev/od chain pair; the PE then permutes the
            # [ev_all; od_all] staging into head-contiguous q^T/k^T via
            # permutation-matrix matmuls (perm tables shipped in pm).
            xstore = {}

            def _load_x_tiles(p):
                xts = []
                for tl in range(4):
                    xt = x_pool.tile([128, C], bf16, tag="x", name=f"x{p}{tl}")
                    t0 = p * 512 + tl * 128
                    for h in range(2):
                        nc.sync.dma_start(
                            xt[:, h * 512:(h + 1) * 512],
                            x_d[t0:t0 + 128, h * 512:(h + 1) * 512])
                    xts.append(xt)
                xstore[p] = xts

            def era1_chunks(p):
                state = {}

                def load_x(p=p):
                    if p == 0:
                        _load_x_tiles(0)
                        for kc in range(NKC):
                            nc.sync.dma_start(w_sb[:, kc, :],
                                              w_d[kc * 128:(kc + 1) * 128, :])
                        nc.sync.dma_start(cs_sb[:], cs_d[:])
                        nc.sync.dma_start(sn_sb[:], sn_d[:])
                        nc.sync.dma_start(pm_sb[:], pm_d[:])
                    elif p == 1:
                        nc.sync.dma_start(
                            wo_sb[:],
                            wo_d[:].rearrange("(kc p) c -> p kc c", p=128).bitcast(f32r))
                        nc.sync.dma_start(tri_sb[:], tri_d[:])
                    state["xts"] = xstore[p]
                    state["xT"] = xT_pool.tile([128, NKC, 512], bf16, tag="xT",
                                               name=f"xT{p}")

                def prefetch_x(p=p):
                    if p + 1 < NQTR:
                        _load_x_tiles(p + 1)

                def transpose_tl(tl, p=p):
                    def emit():
                        xt = state["xts"][tl]
                        xT_q = state["xT"]
                        for kc4 in range(2):
                            ptr = ps_e1_pool.tile([128, 4, 128], bf16, tag="e1",
                                                  name=f"tr{p}{tl}{kc4}")
                            for j in range(4):
                                kc = 4 * kc4 + j
                                nc.tensor.transpose(
                                    ptr[:, j, :], xt[:, kc * 128:(kc + 1) * 128], identr)
                            nc.vector.tensor_copy(
                                xT_q[:, 4 * kc4:4 * kc4 + 4, tl * 128:(tl + 1) * 128],
                                ptr[:])
                    return emit

                def vproj_tl(tl, p=p):
                    def emit():
                        xT_q = state["xT"]
                        psv = ps_e1_pool.tile([128, 256], f32, tag="e1",
                                              name=f"v{p}{tl}")
                        for kc in range(NKC):
                            nc.tensor.matmul(psv[:], xT_q[:, kc, tl * 128:(tl + 1) * 128],
                                             w_sb[:, kc, 512:768],
                                             start=(kc == 0), stop=(kc == NKC - 1))
                        kt = p * 4 + tl
                        nc.scalar.copy(
                            v4[:, kt, :, 0:64],
                            psv[:].rearrange("p (h c) -> p h c", h=4))
                    return emit

                def qk_even(kind, p=p):
                    def emit():
                        xT_q = state["xT"]
                        ci = 0 if kind == "q" else 2
                        ps = ps_e1_pool.tile([128, 512], f32, tag="e1",
                                             name=f"qe{p}{kind}")
                        for kc in range(NKC):
                            nc.tensor.matmul(ps[:], w_sb[:, kc, ci * 128:(ci + 1) * 128],
                                             xT_q[:, kc, :],
                                             start=(kc == 0), stop=(kc == NKC - 1))
                        cs_c = cs_sb[:, p * 512:(p + 1) * 512]
                        sn_c = sn_sb[:, p * 512:(p + 1) * 512]
                        t1 = rope_pool.tile([128, 512], bf16, tag="t1", name=f"t1_{p}{kind}")
                        t1b = rope_pool.tile([128, 512], bf16, tag="t1b", name=f"t1b_{p}{kind}")
                        nc.vector.tensor_tensor(t1[:], ps[:], cs_c, MUL)
                        nc.vector.tensor_tensor(t1b[:], ps[:], sn_c, MUL)
                        state["t1"], state["t1b"] = t1, t1b
                    return emit

                def qk_odd(kind, p=p):
                    def emit():
                        xT_q = state["xT"]
                        ci = 1 if kind == "q" else 3
                        ps = ps_e1_pool.tile([128, 512], f32, tag="e1",
                                             name=f"qo{p}{kind}")
                        for kc in range(NKC):
                            nc.tensor.matmul(ps[:], w_sb[:, kc, ci * 128:(ci + 1) * 128],
                                             xT_q[:, kc, :],
                                             start=(kc == 0), stop=(kc == NKC - 1))
                        cs_c = cs_sb[:, p * 512:(p + 1) * 512]
                        sn_c = sn_sb[:, p * 512:(p + 1) * 512]
                        t2 = rope_pool.tile([128, 512], bf16, tag="t2", name=f"t2_{p}{kind}")
                        t2b = rope_pool.tile([128, 512], bf16, tag="t2b", name=f"t2b_{p}{kind}")
                        nc.vector.tensor_tensor(t2[:], ps[:], sn_c, MUL)
                        nc.vector.tensor_tensor(t2b[:], ps[:], cs_c, MUL)
                        ev = rope_pool.tile([128, 512], bf16, tag="ev", name=f"ev_{p}{kind}")
                        od = rope_pool.tile([128, 512], bf16, tag="od", name=f"od_{p}{kind}")
                        nc.vector.tensor_tensor(ev[:], state["t1"][:], t2[:], SUB)
                        nc.vector.tensor_tensor(od[:], state["t1b"][:], t2b[:], ADD)
                        state["ev"], state["od"] = ev, od
                    return emit

                def qk_merge(kind, p=p):
                    def emit():
                        ev, od = state["ev"], state["od"]
                        dst = qT if kind == "q" else kT
                        cl = slice(p * 512, (p + 1) * 512)
                        for hp in range(2):
                            psM = ps_e1_pool.tile([128, 512], f32, tag="e1",
                                                  name=f"mg{p}{kind}{hp}")
                            nc.tensor.matmul(psM[:], pm_sb[:, 2 * hp, :], ev[:],
                                             start=True, stop=False, skip_group_check=True)
                            nc.tensor.matmul(psM[:], pm_sb[:, 2 * hp + 1, :], od[:],
                                             start=False, stop=True, skip_group_check=True)
                            nc.vector.tensor_copy(dst[hp][:, cl], psM[:])
                    return emit

                chunks = [load_x]
                for tl in range(2):
                    chunks.append(transpose_tl(tl))
                chunks.append(transpose_tl(2))
                chunks.append(vproj_tl(0))
                chunks.append(transpose_tl(3))
                chunks.append(prefetch_x)
                chunks.append(vproj_tl(1))
                chunks.append(vproj_tl(2))
                chunks.append(vproj_tl(3))
                for kind in ("q", "k"):
                    chunks.append(qk_even(kind))
                    chunks.append(qk_odd(kind))
                    chunks.append(qk_merge(kind))
                return chunks

            # ---------------- attention emitters for query-quarter qc ----------------
            def att_blocks(qc):
                blocks = []
                nkt_q = (qc + 1) * 4
                ps_o_tiles = {}

                def sblock(hp, ki2, qc=qc):
                    def emit():
                        nkt_q_ = (qc + 1) * 4
                        ki0 = 2 * ki2
                        if ki2 == 0:
                            ps_o_tiles[hp] = [
                                ps_o_pool.tile([65, 512], f32, tag=f"o{hh}",
                                               name=f"pso{qc}{hp}{hh}")
                                for hh in range(2)]
                        ps_o = ps_o_tiles[hp]
                        ps_s2 = [ps_s_pool.tile([128, 1024], f32, tag="s",
                                                name=f"ps_s{qc}{hp}{ki2}{hh}")
                                 for hh in range(2)]
                        for half in range(2):
                            ki = ki0 + half
                            soff = max(0, ki * 128 - qc * 512) if ki // 4 == qc else 0
                            for hh in range(2):
                                r0 = hh * 64
                                nc.tensor.matmul(
                                    ps_s2[hh][:, half * 512 + soff:(half + 1) * 512],
                                    kT[hp][r0:r0 + 64, ki * 128:(ki + 1) * 128],
                                    qT[hp][r0:r0 + 64, qc * 512 + soff:(qc + 1) * 512],
                                    start=True, stop=True, skip_group_check=True)
                        diag_pair = (ki0 // 4 == qc) or ((ki0 + 1) // 4 == qc)
                        pts = []
                        for hh in range(2):
                            ps_s = ps_s2[hh]
                            ps_sv = ps_s[:].rearrange("p (two f) -> p two f", two=2)
                            pt = pt_pool.tile([128, 2, 512], bf16, tag="pt",
                                              name=f"pt{qc}{hp}{ki2}{hh}")
                            pts.append(pt)
                            if not diag_pair:
                                nc.scalar.activation(pt[:], ps_sv, Exp, scale=0.125, bias=ebias)
                            else:
                                for half in range(2):
                                    ki = ki0 + half
                                    off = ki * 128 - qc * 512
                                    if ki // 4 == qc:   # diagonal tile
                                        # cols < off are never read by the AV
                                        # matmul (it starts at soff), so only
                                        # exp the live region + tri-mask the
                                        # 128-wide diagonal band.
                                        nc.scalar.activation(
                                            pt[:, half, off:512],
                                            ps_sv[:, half, off:512], Exp, scale=0.125, bias=ebias)
                                        nc.gpsimd.tensor_tensor(
                                            pt[:, half, off:off + 128],
                                            pt[:, half, off:off + 128],
                                            tri_sb[:], MUL)
                                    else:
                                        nc.scalar.activation(
                                            pt[:, half, :],
                                            ps_sv[:, half, :], Exp, scale=0.125, bias=ebias)
                        for hh in range(2):
                            h = hp * 2 + hh
                            for half in range(2):
                                ki = ki0 + half
                                soff = (max(0, ki * 128 - qc * 512)
                                        if (ki // 4 == qc and ki != 0) else 0)
                                nc.tensor.matmul(
                                    ps_o[hh][:, soff:512],
                                    v4[:, ki, h, 0:65],
                                    pts[hh][:, half, soff:512],
                                    start=(ki == 0), stop=(ki == nkt_q_ - 1),
                                    skip_group_check=True)
                    return emit

                def norm(hp, qc=qc):
                    def emit():
                        for hh in range(2):
                            ps_o = ps_o_tiles[hp][hh]
                            rrow = nrm_pool.tile([1, 512], f32, tag="rrow",
                                                 name=f"rr{qc}{hp}{hh}")
                            nc.vector.reciprocal(rrow[:], ps_o[64:65, :])
                            bsum = nrm_pool.tile([64, 512], f32, tag="bsum",
                                                 name=f"bs{qc}{hp}{hh}")
                            nc.sync.dma_start(
                                bsum[:], rrow[0:1, None, :].to_broadcast([1, 64, 512]))
                            nc.vector.tensor_tensor(
                                a_sb[hp][hh * 64:(hh + 1) * 64, qc * 512:(qc + 1) * 512],
                                ps_o[0:64, :], bsum[:], MUL)
                    return emit

                def outproj(tl, qc=qc):
                    def emit():
                        ti = qc * 4 + tl
                        for ncol in range(2):
                            psy = ps_e1_pool.tile([128, 512], f32, tag="e1",
                                                  name=f"y{qc}{tl}{ncol}")
                            for kc2 in range(2):
                                nc.tensor.matmul(psy[:], a_sb[kc2][:, ti * 128:(ti + 1) * 128],
                                                 wo_sb[:, kc2, ncol * 512:(ncol + 1) * 512],
                                                 start=(kc2 == 0), stop=(kc2 == 1))
                            yt = y_pool.tile([128, 512], f32, tag="yt",
                                             name=f"yt{qc}{tl}{ncol}")
                            nc.vector.tensor_copy(yt[:], psy[:])
                            nc.sync.dma_start(
                                y_d[ti * 128:(ti + 1) * 128, ncol * 512:(ncol + 1) * 512],
                                yt[:])
                    return emit

                for hp in range(2):
                    for ki2 in range(nkt_q // 2):
                        blocks.append(sblock(hp, ki2))
                    blocks.append(norm(hp))
                outs = [outproj(tl) for tl in range(4)]
                return blocks, outs

            # ---------------- fused stage schedule ----------------
            # stage s runs: era1(s), attention(s-1), out-projection(s-2)
            pending_outs = []
            for stage in range(NQTR + 2):
                e1 = era1_chunks(stage) if (stage < NQTR and 1 in phases) else []
                if stage >= 1 and stage <= NQTR and 2 in phases:
                    a2, outs = att_blocks(stage - 1)
                else:
                    a2, outs = [], []
                a2 = pending_outs + a2
                pending_outs = outs
                if e1:
                    e1[0]()      # x DMAs (and stage-0 weight loads) first
                    e1 = e1[1:]
                _weave(e1, a2)

    if split_waits:
        _split_multi_waits(nc)
    return nc


# ---------------------------------------------------------------------------
# Host-side sharding / gather
# ---------------------------------------------------------------------------

def _rope_tables():
    inv_freq = (1.0 / (ROPE_BASE ** (np.arange(0, D, 2, dtype=np.float32) / D))).astype(np.float32)
    ang = (np.arange(T, dtype=np.float32)[:, None] * inv_freq[None, :]).astype(np.float32)  # (T, 32)
    cos, sin = np.cos(ang), np.sin(ang)
    idx = np.arange(128) % 32
    return np.ascontiguousarray(cos[:, idx].T), np.ascontiguousarray(sin[:, idx].T)  # (128, T)


def _perm_cols(g):
    """w_qkv column order for core group g: [QE|QO|KE|KO|V] (ev/od chains)."""
    cols = []
    for base, par in ((0, 0), (0, 1), (C, 0), (C, 1)):      # QE, QO, KE, KO
        for hl in range(4):
            hg = g * 4 + hl
            for i in range(32):
                cols.append(base + hg * 64 + 2 * i + par)
    for hl in range(4):
        hg = g * 4 + hl
        for d_ in range(64):
            cols.append(2 * C + hg * 64 + d_)
    return np.asarray(cols)


def _perm_tables():
    """pm[:, 2*hp+eo, :]: permutation stationaries mapping ev/od staging rows
    (4 heads x 32) onto head-pair tile rows [hA ev|hA od|hB ev|hB od]."""
    pm = np.zeros((128, 4, 128), np.float32)
    for hp in range(2):
        for j in range(128):
            blk, i = j // 32, j % 32
            h = 2 * hp + (1 if blk >= 2 else 0)
            src_row = h * 32 + i
            if blk % 2 == 0:     # ev rows of the pair tile
                pm[src_row, 2 * hp, j] = 1.0
            else:                # od rows
                pm[src_row, 2 * hp + 1, j] = 1.0
    return pm


def _perm_wo_rows(g):
    # O^T rows follow v's ORIGINAL dim order (RoPE only permutes q/k dims),
    # so the out-projection rows are just this head group's contiguous block.
    return np.arange(g * 256, (g + 1) * 256)


def make_in_maps(x, w_qkv, w_out):
    x = np.asarray(x, np.float32)
    w_qkv = np.asarray(w_qkv, np.float32)
    w_out = np.asarray(w_out, np.float32)
    import concourse.mybir as mybir
    cs, sn = _rope_tables()
    f8np = mybir.dt.np(mybir.dt.float8e4)
    bf16np = mybir.dt.np(mybir.dt.bfloat16)
    tri = np.tril(np.ones((128, 128), np.float32)).T.astype(bf16np)  # tri[k,q]=1 iff q>=k
    pm = _perm_tables().astype(bf16np)
    in_maps = []
    for c in range(NCORES):
        b, g = c // 4, c % 4
        in_maps.append({
            "x": np.ascontiguousarray(x[b]).astype(bf16np),
            "w": np.ascontiguousarray(w_qkv[:, _perm_cols(g)]).astype(bf16np),
            "wo": np.ascontiguousarray(w_out[_perm_wo_rows(g), :]),
            "cs": cs, "sn": sn, "tri": tri, "pm": pm,
        })
    return in_maps


def kernel(x, w_qkv, w_out):
    global _BUILT
    from concourse.bass_utils import run_bass_kernel_spmd

    if _BUILT is None:
        _BUILT = build_nc()
    in_maps = make_in_maps(x, w_qkv, w_out)
    res = run_bass_kernel_spmd(_BUILT, in_maps, core_ids=list(range(NCORES)))
    out = np.zeros((B, T, C), np.float32)
    for c in range(NCORES):
        out[c // 4] += res.results[c]["y"]
    return out
